# revision 14
# baseline (speedup 1.0000x reference)
"""Trainium2 Bass kernel for nn_NestedNarx: batched NARX MLP over basins.

Math (from the reference), for t >= 3:
  h = relu(W_in xt + b_in)          xt = 24 distinct delayed features
  a = tanh(W_ih h + b_ih + b_hh)
  y = W_out a + b_out
t < 3: y = x[t, :, 7].

Design (v2, PE tile_position packing + bf16):
  - All matmul operands bf16; PSUM stays fp32 (TRN2).
  - L1: per 4-basin "quad", four concurrent 32x64 tiled matmuls at
    positions (32r, 64*(r%2)); K rows = 24 features + ones row (bias
    folded into weights).  2 psH banks per quad, 2 basins each.
  - L2: 64x64 quadrant matmuls, 4 concurrent across 2 psZ halves of a
    [128,1024] two-bank tile (4 basins per tile).
  - L3: per basin-pair one 128x32 matmul (w_out placed at columns 2j,
    2j+1), 4 column-slots rotate -> one dense [128,512] psY bank per
    chunk holding y for all 128 basins x 512 t.
  - Evacuation (the bottleneck): DVE does relu for ~6/7 of psH tiles +
    psY bias; ACT does all tanh (FD=1024) + 1/7 of psH relu.

Sharding: pure data-parallel, 8 cores x 128 basins.
"""

import os
import sys

import numpy as np

for _p in ("/opt/trn_rl_repo",):
    if _p not in sys.path and os.path.isdir(_p):
        sys.path.insert(0, _p)

import ml_dtypes
import concourse.bass as bass
import concourse.mybir as mybir
from concourse.tile import TileContext

F32 = mybir.dt.float32
BF16 = mybir.dt.bfloat16
AF = mybir.ActivationFunctionType

T = 4096
NG_ALL = 1024
NCORES = 8
G_CORE = NG_ALL // NCORES  # 128 basins per core
NQUAD = 32                 # quads of 4 basins
CH = 512
NCHUNK = T // CH
HID = 64
ACT_H_EVERY = 7            # every 7th psH tile evacuated on ScalarE


def _split_multiwaits(nc):
    """Single sem-wait per instruction (container walrus limitation)."""
    uid = [0]
    for fn in nc.m.functions:
        for bb in fn.blocks:
            new = []
            for inst in bb.instructions:
                si = inst.sync_info
                waits = list(si.on_wait) if si is not None and si.on_wait else []
                if len(waits) > 1:
                    for w in waits[:-1]:
                        uid[0] += 1
                        new.append(
                            mybir.InstNoOp(
                                name=f"{inst.name}-sw{uid[0]}",
                                engine=inst.engine,
                                bass_nofuse=True,
                                sync_info=mybir.SyncInfo(on_wait=[w], on_update=[]),
                            )
                        )
                    si.on_wait = waits[-1:]
                new.append(inst)
            bb.instructions = new


def build_nc():
    nc = bass.Bass()
    xq = nc.declare_dram_parameter("xq", [NQUAD, 128, T], BF16, isOutput=False)
    w1 = nc.declare_dram_parameter("w1", [128, HID], BF16, isOutput=False)
    w2 = nc.declare_dram_parameter("w2", [128, HID], BF16, isOutput=False)
    w3 = nc.declare_dram_parameter("w3", [16, 128, 32], BF16, isOutput=False)
    b2 = nc.declare_dram_parameter("b2", [128, 1], F32, isOutput=False)
    bo = nc.declare_dram_parameter("bo", [128, 1], F32, isOutput=False)
    y = nc.declare_dram_parameter("y", [NCHUNK, 128, CH], F32, isOutput=True)

    with TileContext(nc) as tc:
        with (
            tc.tile_pool(name="const", bufs=1) as constp,
            tc.tile_pool(name="xs", bufs=6) as xsp,
            tc.tile_pool(name="hh", bufs=8) as hp,
            tc.tile_pool(name="aa", bufs=44) as ap_,
            tc.tile_pool(name="yout", bufs=2) as youtp,
            tc.tile_pool(name="psh", bufs=4, space=bass.MemorySpace.PSUM) as pshp,
            tc.tile_pool(name="psz", bufs=2, space=bass.MemorySpace.PSUM) as pszp,
        ):
            # critical-path weights first so the first quads start ASAP;
            # w3 (L3-only, needed 3+ stages later) trails behind.
            w1t = constp.tile([128, HID], BF16, name="w1t")
            nc.sync.dma_start(out=w1t, in_=w1[:])
            w2t = constp.tile([128, HID], BF16, name="w2t")
            nc.sync.dma_start(out=w2t, in_=w2[:])
            b2t = constp.tile([128, 1], F32, name="b2t")
            nc.sync.dma_start(out=b2t, in_=b2[:])
            bot = constp.tile([128, 1], F32, name="bot")
            nc.sync.dma_start(out=bot, in_=bo[:])
            w3t = {}

            def load_w3():
                for j in range(16):
                    tl = constp.tile([128, 32], BF16, name=f"w3_{j}")
                    nc.sync.dma_start(out=tl, in_=w3[j])
                    w3t[j] = tl

            hctr = [0]

            def h_evac(Ht, psH):
                if hctr[0] % ACT_H_EVERY == ACT_H_EVERY - 1:
                    nc.scalar.activation(Ht, psH, AF.Relu)
                else:
                    nc.vector.tensor_scalar(
                        Ht, psH, 0.0, None, mybir.AluOpType.max
                    )
                hctr[0] += 1

            # Software-pipelined stages over global quad index Q:
            #   S0(Q): DMA xs + L1 -> psH
            #   S1(Q): relu evac psH -> H (DVE / every-7th ACT)
            #   S2(Q): L2 quadrants -> psZ, tanh -> A (ACT)
            #   S3(Q): collect A tiles; chunk's L3 runs as a deferred burst
            #          during the next chunk, psY borrowed from the psh pool
            NQ = NCHUNK * NQUAD
            st = {}
            l3_pending = []   # [ck, items, psY, emitted]
            L3_PER_ITER = 64

            def s0(Q):
                ck, q = Q // NQUAD, Q % NQUAD
                t0 = ck * CH
                xs = xsp.tile([128, CH], BF16, name="xs", tag="xs")
                nc.sync.dma_start(out=xs, in_=xq[q][:, t0 : t0 + CH])
                psH = [
                    pshp.tile([128, CH], F32, name="psH", tag="psH")
                    for _ in range(2)
                ]
                for r in range(4):
                    c = r % 2
                    nc.tensor.matmul(
                        psH[r // 2][64 * c : 64 * c + 64, :],
                        w1t[32 * r : 32 * r + 32, :],
                        xs[32 * r : 32 * r + 32, :],
                        start=True,
                        stop=True,
                        tile_position=(32 * r, 64 * c),
                        skip_group_check=True,
                    )
                st[Q] = {"psH": psH}

            def s1(Q):
                psH = st[Q].pop("psH")
                Hts = []
                for i in range(2):
                    Ht = hp.tile([128, CH], BF16, name="H", tag="H")
                    h_evac(Ht, psH[i])
                    Hts.append(Ht)
                st[Q]["H"] = Hts

            def s2(Q):
                Hts = st[Q].pop("H")
                psZ = pszp.tile([128, 2 * CH], F32, name="psZ", tag="psZ")
                nc.tensor.matmul(
                    psZ[0:64, 0:CH], w2t[0:64], Hts[0][0:64],
                    start=True, stop=True, tile_position=(0, 0),
                    skip_group_check=True,
                )
                nc.tensor.matmul(
                    psZ[64:128, 0:CH], w2t[64:128], Hts[0][64:128],
                    start=True, stop=True, tile_position=(64, 64),
                    skip_group_check=True,
                )
                nc.tensor.matmul(
                    psZ[64:128, CH : 2 * CH], w2t[0:64], Hts[1][0:64],
                    start=True, stop=True, tile_position=(0, 64),
                    skip_group_check=True,
                )
                nc.tensor.matmul(
                    psZ[0:64, CH : 2 * CH], w2t[64:128], Hts[1][64:128],
                    start=True, stop=True, tile_position=(64, 0),
                    skip_group_check=True,
                )
                At = ap_.tile([128, 2 * CH], BF16, name="A", tag="A")
                nc.scalar.activation(At, psZ, AF.Tanh, bias=b2t)
                st[Q]["A"] = At

            chunk_acc = []

            def s3(Q):
                ck, q = Q // NQUAD, Q % NQUAD
                At = st.pop(Q)["A"]
                chunk_acc.append(At)
                if q == NQUAD - 1:
                    l3_pending.append([ck, list(chunk_acc), None, 0])
                    chunk_acc.clear()

            def drain_l3(budget):
                if not l3_pending:
                    return
                ent = l3_pending[0]
                ck, tiles, psY, emitted = ent
                if psY is None:
                    psY = pshp.tile([128, CH], F32, name="psY", tag="psH")
                    ent[2] = psY
                n = 0
                while n < budget and ent[3] < 2 * NQUAD:
                    p = ent[3]
                    q, half = p // 2, p % 2
                    c, j = p % 4, p // 4
                    nc.tensor.matmul(
                        psY[32 * c : 32 * c + 32, :],
                        w3t[j],
                        tiles[q][:, CH * half : CH * half + CH],
                        start=(p < 4),
                        stop=(p == 2 * NQUAD - 1),
                        tile_position=(0, 32 * c),
                        skip_group_check=True,
                    )
                    ent[3] += 1
                    n += 1
                if ent[3] == 2 * NQUAD:
                    ysb = youtp.tile([128, CH], F32, name="ysb", tag="ysb")
                    nc.vector.tensor_scalar(
                        ysb, psY, bot, None, mybir.AluOpType.add
                    )
                    nc.sync.dma_start(out=y[ck], in_=ysb)
                    l3_pending.pop(0)

            for t in range(NQ + 16):
                if t < NQ:
                    s0(t)
                if t == 2:
                    load_w3()
                if 0 <= t - 1 < NQ:
                    s1(t - 1)
                if 0 <= t - 2 < NQ:
                    s2(t - 2)
                drain_l3(L3_PER_ITER)
                if 0 <= t - 3 < NQ:
                    s3(t - 3)
            while l3_pending:
                drain_l3(L3_PER_ITER)
    _split_multiwaits(nc)
    return nc


def _to_bf16(a):
    """fp32 -> bf16 round-to-nearest-even, fast numpy bit trick."""
    u = np.ascontiguousarray(a, np.float32).view(np.uint32)
    r = ((u >> 16) & 1) + np.uint32(0x7FFF)
    return ((u + r) >> 16).astype(np.uint16).view(ml_dtypes.bfloat16)


def prep_weights(W_in, b_in, W_ih, b_ih, b_hh, W_out, b_out):
    W_in = np.asarray(W_in, np.float32)
    A = np.zeros((3, HID, 8), np.float32)
    A[0, :, 0:7] = W_in[:, 0:7] + W_in[:, 21:28]
    A[0, :, 7] = W_in[:, 28] + W_in[:, 31]
    A[1, :, 0:7] = W_in[:, 14:21]
    A[1, :, 7] = W_in[:, 30]
    A[2, :, 0:7] = W_in[:, 7:14]
    A[2, :, 7] = W_in[:, 29]

    w1b = np.zeros((32, HID), np.float32)
    for d in range(3):
        w1b[8 * d : 8 * d + 8] = A[d].T
    w1b[24] = np.asarray(b_in, np.float32)
    w1 = np.tile(w1b, (4, 1))  # [128, 64]

    w2 = np.tile(np.asarray(W_ih, np.float32).T, (2, 1))  # [128, 64]

    wo = np.asarray(W_out, np.float32)[0]  # [64]
    w3 = np.zeros((16, 128, 32), np.float32)
    for j in range(16):
        w3[j, 0:64, 2 * j] = wo
        w3[j, 64:128, 2 * j + 1] = wo

    bb = np.asarray(b_ih, np.float32) + np.asarray(b_hh, np.float32)
    b2 = np.concatenate([bb, bb]).reshape(128, 1).astype(np.float32)
    bo = np.full((128, 1), np.asarray(b_out, np.float32)[0], np.float32)
    return _to_bf16(w1), _to_bf16(w2), _to_bf16(w3), b2, bo


def prep_x_core(x, core):
    """x [4096,1024,8] f32 -> xq [32, 128, 4096] bf16 delay-stacked.

    Partition 32*b + row; rows 0-23: feature 8*(d-1)+c = x[t-d, g, c];
    row 24 = 1.0 (bias); rows 25-31 = 0.  g = 4*q + b.
    """
    xc = np.asarray(x[:, core * G_CORE : (core + 1) * G_CORE, :], np.float32)
    xg = np.ascontiguousarray(xc.transpose(1, 2, 0))  # [128, 8, T]
    out = np.zeros((NQUAD, 4, 32, T), np.float32)
    src = xg.reshape(NQUAD, 4, 8, T)
    for d in (1, 2, 3):
        out[:, :, 8 * (d - 1) : 8 * d, d:] = src[:, :, :, : T - d]
    out[:, :, 24, :] = 1.0
    return _to_bf16(out).reshape(NQUAD, 128, T)


def _basin_of_row():
    """Map psY row -> local basin index."""
    m = np.zeros(128, np.int64)
    for row in range(128):
        c, within = row // 32, row % 32
        j, e = within // 2, within % 2
        p = 4 * j + c
        q, half = p // 2, p % 2
        if half == 0:
            m[row] = 4 * q + e          # (A, B)
        else:
            m[row] = 4 * q + 3 - e      # (D, C)
    return m


_NC_CACHE = {}


def _get_nc():
    if "nc" not in _NC_CACHE:
        _NC_CACHE["nc"] = build_nc()
    return _NC_CACHE["nc"]


def kernel(x, W_in, b_in, W_ih, b_ih, W_hh, b_hh, W_out, b_out, _trace=False):
    from concourse.bass_utils import run_bass_kernel_spmd

    x = np.asarray(x, np.float32)
    w1, w2, w3, b2, bo = prep_weights(W_in, b_in, W_ih, b_ih, b_hh, W_out, b_out)
    in_maps = []
    for core in range(NCORES):
        in_maps.append(
            {
                "xq": prep_x_core(x, core),
                "w1": w1,
                "w2": w2,
                "w3": w3,
                "b2": b2,
                "bo": bo,
            }
        )
    nc = _get_nc()
    res = run_bass_kernel_spmd(nc, in_maps, list(range(NCORES)), trace=_trace)
    _NC_CACHE["last_result"] = res

    rowmap = _basin_of_row()
    out = np.empty((T, NG_ALL, 1), np.float32)
    out[:3, :, 0] = x[:3, :, 7]
    for core in range(NCORES):
        yc = res.results[core]["y"]  # [NCHUNK, 128, CH]
        yflat = yc.transpose(1, 0, 2).reshape(128, T)  # [row, t]
        g0 = core * G_CORE
        out[3:, g0 + rowmap, 0] = yflat[:, 3:].T
    return out


# revision 17
# speedup vs baseline: 1.2016x; 1.2016x over previous
"""Trainium2 Bass kernel for nn_NestedNarx: batched NARX MLP over basins.

Math (from the reference), for t >= 3:
  h = relu(W_in xt + b_in)          xt = 24 distinct delayed features
  a = tanh(W_ih h + b_ih + b_hh)
  y = W_out a + b_out
t < 3: y = x[t, :, 7].

Design (v2, PE tile_position packing + bf16):
  - All matmul operands bf16; PSUM stays fp32 (TRN2).
  - L1: per 4-basin "quad", four concurrent 32x64 tiled matmuls at
    positions (32r, 64*(r%2)); K rows = 24 features + ones row (bias
    folded into weights).  2 psH banks per quad, 2 basins each.
  - L2: 64x64 quadrant matmuls, 4 concurrent across 2 psZ halves of a
    [128,1024] two-bank tile (4 basins per tile).
  - L3: per basin-pair one 128x32 matmul (w_out placed at columns 2j,
    2j+1), 4 column-slots rotate -> one dense [128,512] psY bank per
    chunk holding y for all 128 basins x 512 t.
  - Evacuation (the bottleneck): DVE does relu for ~6/7 of psH tiles +
    psY bias; ACT does all tanh (FD=1024) + 1/7 of psH relu.

Sharding: pure data-parallel, 8 cores x 128 basins.
"""

import os
import sys

import numpy as np

for _p in ("/opt/trn_rl_repo",):
    if _p not in sys.path and os.path.isdir(_p):
        sys.path.insert(0, _p)

import ml_dtypes
import concourse.bass as bass
import concourse.mybir as mybir
from concourse.tile import TileContext

F32 = mybir.dt.float32
BF16 = mybir.dt.bfloat16
AF = mybir.ActivationFunctionType

T = 4096
NG_ALL = 1024
NCORES = 8
G_CORE = NG_ALL // NCORES  # 128 basins per core
NQUAD = 32                 # quads of 4 basins
CH = 512
NCHUNK = T // CH
HID = 64
ACT_H_EVERY = 9            # every 9th psH tile evacuated on ScalarE


def _split_multiwaits(nc):
    """Single sem-wait per instruction (container walrus limitation)."""
    uid = [0]
    for fn in nc.m.functions:
        for bb in fn.blocks:
            new = []
            for inst in bb.instructions:
                si = inst.sync_info
                waits = list(si.on_wait) if si is not None and si.on_wait else []
                if len(waits) > 1:
                    for w in waits[:-1]:
                        uid[0] += 1
                        new.append(
                            mybir.InstNoOp(
                                name=f"{inst.name}-sw{uid[0]}",
                                engine=inst.engine,
                                bass_nofuse=True,
                                sync_info=mybir.SyncInfo(on_wait=[w], on_update=[]),
                            )
                        )
                    si.on_wait = waits[-1:]
                new.append(inst)
            bb.instructions = new


def build_nc():
    nc = bass.Bass()
    xq = nc.declare_dram_parameter("xq", [NQUAD, 128, T], BF16, isOutput=False)
    w1 = nc.declare_dram_parameter("w1", [128, HID], BF16, isOutput=False)
    w2 = nc.declare_dram_parameter("w2", [128, HID], BF16, isOutput=False)
    w3 = nc.declare_dram_parameter("w3", [16, 128, 32], BF16, isOutput=False)
    b2 = nc.declare_dram_parameter("b2", [128, 1], F32, isOutput=False)
    bo = nc.declare_dram_parameter("bo", [128, 1], F32, isOutput=False)
    y = nc.declare_dram_parameter("y", [NCHUNK, 128, CH], F32, isOutput=True)

    with TileContext(nc) as tc:
        with (
            tc.tile_pool(name="const", bufs=1) as constp,
            tc.tile_pool(name="xs", bufs=6) as xsp,
            tc.tile_pool(name="hh", bufs=8) as hp,
            tc.tile_pool(name="aa", bufs=44) as ap_,
            tc.tile_pool(name="yout", bufs=2) as youtp,
            tc.tile_pool(name="psh", bufs=4, space=bass.MemorySpace.PSUM) as pshp,
            tc.tile_pool(name="psz", bufs=2, space=bass.MemorySpace.PSUM) as pszp,
        ):
            # critical-path weights first so the first quads start ASAP;
            # w3 (L3-only, needed 3+ stages later) trails behind.
            w1t = constp.tile([128, HID], BF16, name="w1t")
            nc.sync.dma_start(out=w1t, in_=w1[:])
            w2t = constp.tile([128, HID], BF16, name="w2t")
            nc.sync.dma_start(out=w2t, in_=w2[:])
            b2t = constp.tile([128, 1], F32, name="b2t")
            nc.sync.dma_start(out=b2t, in_=b2[:])
            bot = constp.tile([128, 1], F32, name="bot")
            nc.sync.dma_start(out=bot, in_=bo[:])
            w3t = {}

            def load_w3():
                for j in range(16):
                    tl = constp.tile([128, 32], BF16, name=f"w3_{j}")
                    nc.sync.dma_start(out=tl, in_=w3[j])
                    w3t[j] = tl

            hctr = [0]

            def h_evac(Ht, psH):
                if hctr[0] % ACT_H_EVERY == ACT_H_EVERY - 1:
                    nc.scalar.activation(Ht, psH, AF.Relu)
                else:
                    nc.vector.tensor_scalar(
                        Ht, psH, 0.0, None, mybir.AluOpType.max
                    )
                hctr[0] += 1

            # Software-pipelined stages over global quad index Q:
            #   S0(Q): DMA xs + L1 -> psH
            #   S1(Q): relu evac psH -> H (DVE / every-7th ACT)
            #   S2(Q): L2 quadrants -> psZ, tanh -> A (ACT)
            #   S3(Q): collect A tiles; chunk's L3 runs as a deferred burst
            #          during the next chunk, psY borrowed from the psh pool
            NQ = NCHUNK * NQUAD
            st = {}
            l3_pending = []   # [ck, items, psY, emitted]
            L3_PER_ITER = 32

            def s0(Q):
                ck, q = Q // NQUAD, Q % NQUAD
                t0 = ck * CH
                xs = xsp.tile([128, CH], BF16, name="xs", tag="xs")
                nc.sync.dma_start(out=xs, in_=xq[q][:, t0 : t0 + CH])
                psH = [
                    pshp.tile([128, CH], F32, name="psH", tag="psH")
                    for _ in range(2)
                ]
                for r in range(4):
                    c = r % 2
                    nc.tensor.matmul(
                        psH[r // 2][64 * c : 64 * c + 64, :],
                        w1t[32 * r : 32 * r + 32, :],
                        xs[32 * r : 32 * r + 32, :],
                        start=True,
                        stop=True,
                        tile_position=(32 * r, 64 * c),
                        skip_group_check=True,
                    )
                st[Q] = {"psH": psH}

            def s1(Q):
                psH = st[Q].pop("psH")
                Hts = []
                for i in range(2):
                    Ht = hp.tile([128, CH], BF16, name="H", tag="H")
                    h_evac(Ht, psH[i])
                    Hts.append(Ht)
                st[Q]["H"] = Hts

            def s2(Q):
                Hts = st[Q].pop("H")
                psZ = pszp.tile([128, 2 * CH], F32, name="psZ", tag="psZ")
                nc.tensor.matmul(
                    psZ[0:64, 0:CH], w2t[0:64], Hts[0][0:64],
                    start=True, stop=True, tile_position=(0, 0),
                    skip_group_check=True,
                )
                nc.tensor.matmul(
                    psZ[64:128, 0:CH], w2t[64:128], Hts[0][64:128],
                    start=True, stop=True, tile_position=(64, 64),
                    skip_group_check=True,
                )
                nc.tensor.matmul(
                    psZ[64:128, CH : 2 * CH], w2t[0:64], Hts[1][0:64],
                    start=True, stop=True, tile_position=(0, 64),
                    skip_group_check=True,
                )
                nc.tensor.matmul(
                    psZ[0:64, CH : 2 * CH], w2t[64:128], Hts[1][64:128],
                    start=True, stop=True, tile_position=(64, 0),
                    skip_group_check=True,
                )
                At = ap_.tile([128, 2 * CH], BF16, name="A", tag="A")
                nc.scalar.activation(At, psZ, AF.Tanh, bias=b2t)
                st[Q]["A"] = At

            chunk_acc = []

            def s3(Q):
                ck, q = Q // NQUAD, Q % NQUAD
                At = st.pop(Q)["A"]
                chunk_acc.append(At)
                if q == NQUAD - 1:
                    l3_pending.append([ck, list(chunk_acc), None, 0])
                    chunk_acc.clear()

            def drain_l3(budget):
                if not l3_pending:
                    return
                ent = l3_pending[0]
                ck, tiles, psY, emitted = ent
                if psY is None:
                    psY = pszp.tile([128, CH], F32, name="psY", tag="psZ")
                    ent[2] = psY
                n = 0
                while n < budget and ent[3] < 2 * NQUAD:
                    p = ent[3]
                    q, half = p // 2, p % 2
                    c, j = p % 4, p // 4
                    nc.tensor.matmul(
                        psY[32 * c : 32 * c + 32, :],
                        w3t[j],
                        tiles[q][:, CH * half : CH * half + CH],
                        start=(p < 4),
                        stop=(p == 2 * NQUAD - 1),
                        tile_position=(0, 32 * c),
                        skip_group_check=True,
                    )
                    ent[3] += 1
                    n += 1
                if ent[3] == 2 * NQUAD:
                    ysb = youtp.tile([128, CH], F32, name="ysb", tag="ysb")
                    nc.vector.tensor_scalar(
                        ysb, psY, bot, None, mybir.AluOpType.add
                    )
                    nc.sync.dma_start(out=y[ck], in_=ysb)
                    l3_pending.pop(0)

            for t in range(NQ + 16):
                if t < NQ:
                    s0(t)
                if t == 2:
                    load_w3()
                if 0 <= t - 1 < NQ:
                    s1(t - 1)
                if 0 <= t - 2 < NQ:
                    s2(t - 2)
                drain_l3(L3_PER_ITER)
                if 0 <= t - 3 < NQ:
                    s3(t - 3)
            while l3_pending:
                drain_l3(L3_PER_ITER)
    _split_multiwaits(nc)
    return nc


def _to_bf16(a):
    """fp32 -> bf16 round-to-nearest-even, fast numpy bit trick."""
    u = np.ascontiguousarray(a, np.float32).view(np.uint32)
    r = ((u >> 16) & 1) + np.uint32(0x7FFF)
    return ((u + r) >> 16).astype(np.uint16).view(ml_dtypes.bfloat16)


def prep_weights(W_in, b_in, W_ih, b_ih, b_hh, W_out, b_out):
    W_in = np.asarray(W_in, np.float32)
    A = np.zeros((3, HID, 8), np.float32)
    A[0, :, 0:7] = W_in[:, 0:7] + W_in[:, 21:28]
    A[0, :, 7] = W_in[:, 28] + W_in[:, 31]
    A[1, :, 0:7] = W_in[:, 14:21]
    A[1, :, 7] = W_in[:, 30]
    A[2, :, 0:7] = W_in[:, 7:14]
    A[2, :, 7] = W_in[:, 29]

    w1b = np.zeros((32, HID), np.float32)
    for d in range(3):
        w1b[8 * d : 8 * d + 8] = A[d].T
    w1b[24] = np.asarray(b_in, np.float32)
    w1 = np.tile(w1b, (4, 1))  # [128, 64]

    w2 = np.tile(np.asarray(W_ih, np.float32).T, (2, 1))  # [128, 64]

    wo = np.asarray(W_out, np.float32)[0]  # [64]
    w3 = np.zeros((16, 128, 32), np.float32)
    for j in range(16):
        w3[j, 0:64, 2 * j] = wo
        w3[j, 64:128, 2 * j + 1] = wo

    bb = np.asarray(b_ih, np.float32) + np.asarray(b_hh, np.float32)
    b2 = np.concatenate([bb, bb]).reshape(128, 1).astype(np.float32)
    bo = np.full((128, 1), np.asarray(b_out, np.float32)[0], np.float32)
    return _to_bf16(w1), _to_bf16(w2), _to_bf16(w3), b2, bo


def prep_x_core(x, core):
    """x [4096,1024,8] f32 -> xq [32, 128, 4096] bf16 delay-stacked.

    Partition 32*b + row; rows 0-23: feature 8*(d-1)+c = x[t-d, g, c];
    row 24 = 1.0 (bias); rows 25-31 = 0.  g = 4*q + b.
    """
    xc = np.asarray(x[:, core * G_CORE : (core + 1) * G_CORE, :], np.float32)
    xg = np.ascontiguousarray(xc.transpose(1, 2, 0))  # [128, 8, T]
    out = np.zeros((NQUAD, 4, 32, T), np.float32)
    src = xg.reshape(NQUAD, 4, 8, T)
    for d in (1, 2, 3):
        out[:, :, 8 * (d - 1) : 8 * d, d:] = src[:, :, :, : T - d]
    out[:, :, 24, :] = 1.0
    return _to_bf16(out).reshape(NQUAD, 128, T)


def _basin_of_row():
    """Map psY row -> local basin index."""
    m = np.zeros(128, np.int64)
    for row in range(128):
        c, within = row // 32, row % 32
        j, e = within // 2, within % 2
        p = 4 * j + c
        q, half = p // 2, p % 2
        if half == 0:
            m[row] = 4 * q + e          # (A, B)
        else:
            m[row] = 4 * q + 3 - e      # (D, C)
    return m


_NC_CACHE = {}


def _get_nc():
    if "nc" not in _NC_CACHE:
        _NC_CACHE["nc"] = build_nc()
    return _NC_CACHE["nc"]


def kernel(x, W_in, b_in, W_ih, b_ih, W_hh, b_hh, W_out, b_out, _trace=False):
    from concourse.bass_utils import run_bass_kernel_spmd

    x = np.asarray(x, np.float32)
    w1, w2, w3, b2, bo = prep_weights(W_in, b_in, W_ih, b_ih, b_hh, W_out, b_out)
    in_maps = []
    for core in range(NCORES):
        in_maps.append(
            {
                "xq": prep_x_core(x, core),
                "w1": w1,
                "w2": w2,
                "w3": w3,
                "b2": b2,
                "bo": bo,
            }
        )
    nc = _get_nc()
    res = run_bass_kernel_spmd(nc, in_maps, list(range(NCORES)), trace=_trace)
    _NC_CACHE["last_result"] = res

    rowmap = _basin_of_row()
    out = np.empty((T, NG_ALL, 1), np.float32)
    out[:3, :, 0] = x[:3, :, 7]
    for core in range(NCORES):
        yc = res.results[core]["y"]  # [NCHUNK, 128, CH]
        yflat = yc.transpose(1, 0, 2).reshape(128, T)  # [row, t]
        g0 = core * G_CORE
        out[3:, g0 + rowmap, 0] = yflat[:, 3:].T
    return out


# revision 21
# speedup vs baseline: 1.2336x; 1.0266x over previous
"""Trainium2 Bass kernel for nn_NestedNarx: batched NARX MLP over basins.

Math (from the reference), for t >= 3:
  h = relu(W_in xt + b_in)          xt = 24 distinct delayed features
  a = tanh(W_ih h + b_ih + b_hh)
  y = W_out a + b_out
t < 3: y = x[t, :, 7].

Design (v2, PE tile_position packing + bf16):
  - All matmul operands bf16; PSUM stays fp32 (TRN2).
  - L1: per 4-basin "quad", four concurrent 32x64 tiled matmuls at
    positions (32r, 64*(r%2)); K rows = 24 features + ones row (bias
    folded into weights).  2 psH banks per quad, 2 basins each.
  - L2: 64x64 quadrant matmuls, 4 concurrent across 2 psZ halves of a
    [128,1024] two-bank tile (4 basins per tile).
  - L3: per basin-pair one 128x32 matmul (w_out placed at columns 2j,
    2j+1), 4 column-slots rotate -> one dense [128,512] psY bank per
    chunk holding y for all 128 basins x 512 t.
  - Evacuation (the bottleneck): DVE does relu for ~6/7 of psH tiles +
    psY bias; ACT does all tanh (FD=1024) + 1/7 of psH relu.

Sharding: pure data-parallel, 8 cores x 128 basins.
"""

import os
import sys

import numpy as np

for _p in ("/opt/trn_rl_repo",):
    if _p not in sys.path and os.path.isdir(_p):
        sys.path.insert(0, _p)

import ml_dtypes
import concourse.bass as bass
import concourse.mybir as mybir
from concourse.tile import TileContext

F32 = mybir.dt.float32
BF16 = mybir.dt.bfloat16
AF = mybir.ActivationFunctionType

T = 4096
NG_ALL = 1024
NCORES = 8
G_CORE = NG_ALL // NCORES  # 128 basins per core
NQUAD = 32                 # quads of 4 basins
CH = 512
NCHUNK = T // CH
HID = 64
ACT_H_EVERY = 40           # every 40th psH quad-tile evacuated on ScalarE


def _split_multiwaits(nc):
    """Single sem-wait per instruction (container walrus limitation)."""
    uid = [0]
    for fn in nc.m.functions:
        for bb in fn.blocks:
            new = []
            for inst in bb.instructions:
                si = inst.sync_info
                waits = list(si.on_wait) if si is not None and si.on_wait else []
                if len(waits) > 1:
                    for w in waits[:-1]:
                        uid[0] += 1
                        new.append(
                            mybir.InstNoOp(
                                name=f"{inst.name}-sw{uid[0]}",
                                engine=inst.engine,
                                bass_nofuse=True,
                                sync_info=mybir.SyncInfo(on_wait=[w], on_update=[]),
                            )
                        )
                    si.on_wait = waits[-1:]
                new.append(inst)
            bb.instructions = new


def build_nc():
    nc = bass.Bass()
    xq = nc.declare_dram_parameter("xq", [NQUAD, 128, T], BF16, isOutput=False)
    w1 = nc.declare_dram_parameter("w1", [128, HID], BF16, isOutput=False)
    w2 = nc.declare_dram_parameter("w2", [128, HID], BF16, isOutput=False)
    w3 = nc.declare_dram_parameter("w3", [16, 128, 32], BF16, isOutput=False)
    b2 = nc.declare_dram_parameter("b2", [128, 1], F32, isOutput=False)
    bo = nc.declare_dram_parameter("bo", [128, 1], F32, isOutput=False)
    y = nc.declare_dram_parameter("y", [NCHUNK, 128, CH], F32, isOutput=True)

    with TileContext(nc) as tc:
        with (
            tc.tile_pool(name="const", bufs=1) as constp,
            tc.tile_pool(name="xs", bufs=6) as xsp,
            tc.tile_pool(name="hh", bufs=8) as hp,
            tc.tile_pool(name="aa", bufs=44) as ap_,
            tc.tile_pool(name="yout", bufs=2) as youtp,
            tc.tile_pool(name="psh", bufs=2, space=bass.MemorySpace.PSUM) as pshp,
            tc.tile_pool(name="psz", bufs=2, space=bass.MemorySpace.PSUM) as pszp,
        ):
            # critical-path weights first so the first quads start ASAP;
            # w3 (L3-only, needed 3+ stages later) trails behind.
            w1t = constp.tile([128, HID], BF16, name="w1t")
            nc.sync.dma_start(out=w1t, in_=w1[:])
            w2t = constp.tile([128, HID], BF16, name="w2t")
            nc.sync.dma_start(out=w2t, in_=w2[:])
            b2t = constp.tile([128, 1], F32, name="b2t")
            nc.sync.dma_start(out=b2t, in_=b2[:])
            bot = constp.tile([128, 1], F32, name="bot")
            nc.sync.dma_start(out=bot, in_=bo[:])
            w3t = {}

            def load_w3():
                for j in range(16):
                    tl = constp.tile([128, 32], BF16, name=f"w3_{j}")
                    nc.sync.dma_start(out=tl, in_=w3[j])
                    w3t[j] = tl

            hctr = [0]

            def h_evac(Ht, psH):
                if hctr[0] % ACT_H_EVERY == ACT_H_EVERY - 1:
                    nc.scalar.activation(Ht, psH, AF.Relu)
                else:
                    nc.vector.tensor_scalar(
                        Ht, psH, 0.0, None, mybir.AluOpType.max
                    )
                hctr[0] += 1

            # Software-pipelined stages over global quad index Q:
            #   S0(Q): DMA xs + L1 -> psH
            #   S1(Q): relu evac psH -> H (DVE / every-7th ACT)
            #   S2(Q): L2 quadrants -> psZ, tanh -> A (ACT)
            #   S3(Q): collect A tiles; chunk's L3 runs as a deferred burst
            #          during the next chunk, psY borrowed from the psh pool
            NQ = NCHUNK * NQUAD
            st = {}
            l3_pending = []   # [ck, items, psY, emitted]
            L3_PER_ITER = 32

            # L1 slot layout: 4 basins in one [128,1024] 2-bank psH tile
            #   r=0 (0,0)   -> parts 0:64,   cols 0:CH
            #   r=1 (32,64) -> parts 64:128, cols 0:CH
            #   r=2 (64,0)  -> parts 0:64,   cols CH:2CH
            #   r=3 (96,64) -> parts 64:128, cols CH:2CH
            def s0(Q):
                ck, q = Q // NQUAD, Q % NQUAD
                t0 = ck * CH
                xs = xsp.tile([128, CH], BF16, name="xs", tag="xs")
                nc.sync.dma_start(out=xs, in_=xq[q][:, t0 : t0 + CH])
                psH = pshp.tile([128, 2 * CH], F32, name="psH", tag="psH")
                for r in range(4):
                    pp = 64 * (r % 2)
                    cc = CH * (r // 2)
                    nc.tensor.matmul(
                        psH[pp : pp + 64, cc : cc + CH],
                        w1t[32 * r : 32 * r + 32, :],
                        xs[32 * r : 32 * r + 32, :],
                        start=True,
                        stop=True,
                        tile_position=(32 * r, pp),
                        skip_group_check=True,
                    )
                st[Q] = {"psH": psH}

            def s1(Q):
                psH = st[Q].pop("psH")
                Ht = hp.tile([128, 2 * CH], BF16, name="H", tag="H")
                h_evac(Ht, psH)
                st[Q]["H"] = Ht

            def s2(Q):
                Ht = st[Q].pop("H")
                psZ = pszp.tile([128, 2 * CH], F32, name="psZ", tag="psZ")
                nc.tensor.matmul(
                    psZ[0:64, 0:CH], w2t[0:64], Ht[0:64, 0:CH],
                    start=True, stop=True, tile_position=(0, 0),
                    skip_group_check=True,
                )
                nc.tensor.matmul(
                    psZ[64:128, 0:CH], w2t[64:128], Ht[64:128, 0:CH],
                    start=True, stop=True, tile_position=(64, 64),
                    skip_group_check=True,
                )
                nc.tensor.matmul(
                    psZ[64:128, CH : 2 * CH], w2t[0:64], Ht[0:64, CH : 2 * CH],
                    start=True, stop=True, tile_position=(0, 64),
                    skip_group_check=True,
                )
                nc.tensor.matmul(
                    psZ[0:64, CH : 2 * CH], w2t[64:128], Ht[64:128, CH : 2 * CH],
                    start=True, stop=True, tile_position=(64, 0),
                    skip_group_check=True,
                )
                At = ap_.tile([128, 2 * CH], BF16, name="A", tag="A")
                nc.scalar.activation(At, psZ, AF.Tanh, bias=b2t)
                st[Q]["A"] = At

            chunk_acc = []

            def s3(Q):
                ck, q = Q // NQUAD, Q % NQUAD
                At = st.pop(Q)["A"]
                chunk_acc.append(At)
                if q == NQUAD - 1:
                    l3_pending.append([ck, list(chunk_acc), None, 0])
                    chunk_acc.clear()

            def drain_l3(budget):
                if not l3_pending:
                    return
                ent = l3_pending[0]
                ck, tiles, psY, emitted = ent
                if psY is None:
                    # last chunk drains post-loop when the psh pool is idle
                    pool = pshp if ck == NCHUNK - 1 else pszp
                    tag = "psH" if ck == NCHUNK - 1 else "psZ"
                    psY = pool.tile([128, CH], F32, name="psY", tag=tag)
                    ent[2] = psY
                n = 0
                while n < budget and ent[3] < 2 * NQUAD:
                    p = ent[3]
                    q, half = p // 2, p % 2
                    c, j = p % 4, p // 4
                    nc.tensor.matmul(
                        psY[32 * c : 32 * c + 32, :],
                        w3t[j],
                        tiles[q][:, CH * half : CH * half + CH],
                        start=(p < 4),
                        stop=(p == 2 * NQUAD - 1),
                        tile_position=(0, 32 * c),
                        skip_group_check=True,
                    )
                    ent[3] += 1
                    n += 1
                if ent[3] == 2 * NQUAD:
                    ysb = youtp.tile([128, CH], F32, name="ysb", tag="ysb")
                    nc.vector.tensor_scalar(
                        ysb, psY, bot, None, mybir.AluOpType.add
                    )
                    nc.sync.dma_start(out=y[ck], in_=ysb)
                    l3_pending.pop(0)

            for t in range(NQ + 16):
                if t < NQ:
                    s0(t)
                if t == 2:
                    load_w3()
                if 0 <= t - 1 < NQ:
                    s1(t - 1)
                if 0 <= t - 2 < NQ:
                    s2(t - 2)
                drain_l3(L3_PER_ITER)
                if 0 <= t - 3 < NQ:
                    s3(t - 3)
            while l3_pending:
                drain_l3(L3_PER_ITER)
    _split_multiwaits(nc)
    return nc


def _to_bf16(a):
    """fp32 -> bf16 round-to-nearest-even, fast numpy bit trick."""
    u = np.ascontiguousarray(a, np.float32).view(np.uint32)
    r = ((u >> 16) & 1) + np.uint32(0x7FFF)
    return ((u + r) >> 16).astype(np.uint16).view(ml_dtypes.bfloat16)


def prep_weights(W_in, b_in, W_ih, b_ih, b_hh, W_out, b_out):
    W_in = np.asarray(W_in, np.float32)
    A = np.zeros((3, HID, 8), np.float32)
    A[0, :, 0:7] = W_in[:, 0:7] + W_in[:, 21:28]
    A[0, :, 7] = W_in[:, 28] + W_in[:, 31]
    A[1, :, 0:7] = W_in[:, 14:21]
    A[1, :, 7] = W_in[:, 30]
    A[2, :, 0:7] = W_in[:, 7:14]
    A[2, :, 7] = W_in[:, 29]

    w1b = np.zeros((32, HID), np.float32)
    for d in range(3):
        w1b[8 * d : 8 * d + 8] = A[d].T
    w1b[24] = np.asarray(b_in, np.float32)
    w1 = np.tile(w1b, (4, 1))  # [128, 64]

    w2 = np.tile(np.asarray(W_ih, np.float32).T, (2, 1))  # [128, 64]

    wo = np.asarray(W_out, np.float32)[0]  # [64]
    w3 = np.zeros((16, 128, 32), np.float32)
    for j in range(16):
        w3[j, 0:64, 2 * j] = wo
        w3[j, 64:128, 2 * j + 1] = wo

    bb = np.asarray(b_ih, np.float32) + np.asarray(b_hh, np.float32)
    b2 = np.concatenate([bb, bb]).reshape(128, 1).astype(np.float32)
    bo = np.full((128, 1), np.asarray(b_out, np.float32)[0], np.float32)
    return _to_bf16(w1), _to_bf16(w2), _to_bf16(w3), b2, bo


def prep_x_core(x, core):
    """x [4096,1024,8] f32 -> xq [32, 128, 4096] bf16 delay-stacked.

    Partition 32*b + row; rows 0-23: feature 8*(d-1)+c = x[t-d, g, c];
    row 24 = 1.0 (bias); rows 25-31 = 0.  g = 4*q + b.
    """
    xc = np.asarray(x[:, core * G_CORE : (core + 1) * G_CORE, :], np.float32)
    xg = np.ascontiguousarray(xc.transpose(1, 2, 0))  # [128, 8, T]
    out = np.zeros((NQUAD, 4, 32, T), np.float32)
    src = xg.reshape(NQUAD, 4, 8, T)
    for d in (1, 2, 3):
        out[:, :, 8 * (d - 1) : 8 * d, d:] = src[:, :, :, : T - d]
    out[:, :, 24, :] = 1.0
    return _to_bf16(out).reshape(NQUAD, 128, T)


def _basin_of_row():
    """Map psY row -> local basin index."""
    m = np.zeros(128, np.int64)
    for row in range(128):
        c, within = row // 32, row % 32
        j, e = within // 2, within % 2
        p = 4 * j + c
        q, half = p // 2, p % 2
        if half == 0:
            m[row] = 4 * q + e          # (A, B)
        else:
            m[row] = 4 * q + 3 - e      # (D, C)
    return m


_NC_CACHE = {}


def _get_nc():
    if "nc" not in _NC_CACHE:
        _NC_CACHE["nc"] = build_nc()
    return _NC_CACHE["nc"]


def kernel(x, W_in, b_in, W_ih, b_ih, W_hh, b_hh, W_out, b_out, _trace=False):
    from concourse.bass_utils import run_bass_kernel_spmd

    x = np.asarray(x, np.float32)
    w1, w2, w3, b2, bo = prep_weights(W_in, b_in, W_ih, b_ih, b_hh, W_out, b_out)
    in_maps = []
    for core in range(NCORES):
        in_maps.append(
            {
                "xq": prep_x_core(x, core),
                "w1": w1,
                "w2": w2,
                "w3": w3,
                "b2": b2,
                "bo": bo,
            }
        )
    nc = _get_nc()
    res = run_bass_kernel_spmd(nc, in_maps, list(range(NCORES)), trace=_trace)
    _NC_CACHE["last_result"] = res

    rowmap = _basin_of_row()
    out = np.empty((T, NG_ALL, 1), np.float32)
    out[:3, :, 0] = x[:3, :, 7]
    for core in range(NCORES):
        yc = res.results[core]["y"]  # [NCHUNK, 128, CH]
        yflat = yc.transpose(1, 0, 2).reshape(128, T)  # [row, t]
        g0 = core * G_CORE
        out[3:, g0 + rowmap, 0] = yflat[:, 3:].T
    return out


# revision 23
# speedup vs baseline: 1.2346x; 1.0008x over previous
"""Trainium2 Bass kernel for nn_NestedNarx: batched NARX MLP over basins.

Math (from the reference), for t >= 3:
  h = relu(W_in xt + b_in)          xt = 24 distinct delayed features
  a = tanh(W_ih h + b_ih + b_hh)
  y = W_out a + b_out
t < 3: y = x[t, :, 7].

Design (v2, PE tile_position packing + bf16):
  - All matmul operands bf16; PSUM stays fp32 (TRN2).
  - L1: per 4-basin "quad", four concurrent 32x64 tiled matmuls at
    positions (32r, 64*(r%2)); K rows = 24 features + ones row (bias
    folded into weights).  2 psH banks per quad, 2 basins each.
  - L2: 64x64 quadrant matmuls, 4 concurrent across 2 psZ halves of a
    [128,1024] two-bank tile (4 basins per tile).
  - L3: per basin-pair one 128x32 matmul (w_out placed at columns 2j,
    2j+1), 4 column-slots rotate -> one dense [128,512] psY bank per
    chunk holding y for all 128 basins x 512 t.
  - Evacuation (the bottleneck): DVE does relu for ~6/7 of psH tiles +
    psY bias; ACT does all tanh (FD=1024) + 1/7 of psH relu.

Sharding: pure data-parallel, 8 cores x 128 basins.
"""

import os
import sys

import numpy as np

for _p in ("/opt/trn_rl_repo",):
    if _p not in sys.path and os.path.isdir(_p):
        sys.path.insert(0, _p)

import ml_dtypes
import concourse.bass as bass
import concourse.mybir as mybir
from concourse.tile import TileContext

F32 = mybir.dt.float32
BF16 = mybir.dt.bfloat16
AF = mybir.ActivationFunctionType

T = 4096
NG_ALL = 1024
NCORES = 8
G_CORE = NG_ALL // NCORES  # 128 basins per core
NQUAD = 32                 # quads of 4 basins
CH = 512
NCHUNK = T // CH
HID = 64
ACT_H_EVERY = 19           # every 19th psH quad-tile evacuated on ScalarE


def _split_multiwaits(nc):
    """Single sem-wait per instruction (container walrus limitation)."""
    uid = [0]
    for fn in nc.m.functions:
        for bb in fn.blocks:
            new = []
            for inst in bb.instructions:
                si = inst.sync_info
                waits = list(si.on_wait) if si is not None and si.on_wait else []
                if len(waits) > 1:
                    for w in waits[:-1]:
                        uid[0] += 1
                        new.append(
                            mybir.InstNoOp(
                                name=f"{inst.name}-sw{uid[0]}",
                                engine=inst.engine,
                                bass_nofuse=True,
                                sync_info=mybir.SyncInfo(on_wait=[w], on_update=[]),
                            )
                        )
                    si.on_wait = waits[-1:]
                new.append(inst)
            bb.instructions = new


def build_nc():
    nc = bass.Bass()
    xq = nc.declare_dram_parameter("xq", [NQUAD, 128, T], BF16, isOutput=False)
    w1 = nc.declare_dram_parameter("w1", [128, HID], BF16, isOutput=False)
    w2 = nc.declare_dram_parameter("w2", [128, HID], BF16, isOutput=False)
    w3 = nc.declare_dram_parameter("w3", [16, 128, 32], BF16, isOutput=False)
    b2 = nc.declare_dram_parameter("b2", [128, 1], F32, isOutput=False)
    bo = nc.declare_dram_parameter("bo", [128, 1], F32, isOutput=False)
    y = nc.declare_dram_parameter("y", [NCHUNK, 128, CH], F32, isOutput=True)

    with TileContext(nc) as tc:
        with (
            tc.tile_pool(name="const", bufs=1) as constp,
            tc.tile_pool(name="xs", bufs=6) as xsp,
            tc.tile_pool(name="hh", bufs=8) as hp,
            tc.tile_pool(name="aa", bufs=44) as ap_,
            tc.tile_pool(name="yout", bufs=2) as youtp,
            tc.tile_pool(name="psh", bufs=2, space=bass.MemorySpace.PSUM) as pshp,
            tc.tile_pool(name="psz", bufs=2, space=bass.MemorySpace.PSUM) as pszp,
        ):
            # critical-path weights first so the first quads start ASAP;
            # w3 (L3-only, needed 3+ stages later) trails behind.
            w1t = constp.tile([128, HID], BF16, name="w1t")
            nc.sync.dma_start(out=w1t, in_=w1[:])
            w2t = constp.tile([128, HID], BF16, name="w2t")
            nc.sync.dma_start(out=w2t, in_=w2[:])
            b2t = constp.tile([128, 1], F32, name="b2t")
            nc.sync.dma_start(out=b2t, in_=b2[:])
            bot = constp.tile([128, 1], F32, name="bot")
            nc.sync.dma_start(out=bot, in_=bo[:])
            w3t = {}

            def load_w3():
                for j in range(16):
                    tl = constp.tile([128, 32], BF16, name=f"w3_{j}")
                    nc.sync.dma_start(out=tl, in_=w3[j])
                    w3t[j] = tl

            hctr = [0]

            def h_evac(Ht, psH):
                if hctr[0] % ACT_H_EVERY == ACT_H_EVERY - 1:
                    nc.scalar.activation(Ht, psH, AF.Relu)
                else:
                    nc.vector.tensor_scalar(
                        Ht, psH, 0.0, None, mybir.AluOpType.max
                    )
                hctr[0] += 1

            # Software-pipelined stages over global quad index Q:
            #   S0(Q): DMA xs + L1 -> psH
            #   S1(Q): relu evac psH -> H (DVE / every-7th ACT)
            #   S2(Q): L2 quadrants -> psZ, tanh -> A (ACT)
            #   S3(Q): collect A tiles; chunk's L3 runs as a deferred burst
            #          during the next chunk, psY borrowed from the psh pool
            NQ = NCHUNK * NQUAD
            st = {}
            l3_pending = []   # [ck, items, psY, emitted]
            L3_PER_ITER = 32

            # L1 slot layout: 4 basins in one [128,1024] 2-bank psH tile
            #   r=0 (0,0)   -> parts 0:64,   cols 0:CH
            #   r=1 (32,64) -> parts 64:128, cols 0:CH
            #   r=2 (64,0)  -> parts 0:64,   cols CH:2CH
            #   r=3 (96,64) -> parts 64:128, cols CH:2CH
            def s0(Q):
                ck, q = Q // NQUAD, Q % NQUAD
                t0 = ck * CH
                xs = xsp.tile([128, CH], BF16, name="xs", tag="xs")
                nc.sync.dma_start(out=xs, in_=xq[q][:, t0 : t0 + CH])
                psH = pshp.tile([128, 2 * CH], F32, name="psH", tag="psH")
                for r in range(4):
                    pp = 64 * (r % 2)
                    cc = CH * (r // 2)
                    nc.tensor.matmul(
                        psH[pp : pp + 64, cc : cc + CH],
                        w1t[32 * r : 32 * r + 32, :],
                        xs[32 * r : 32 * r + 32, :],
                        start=True,
                        stop=True,
                        tile_position=(32 * r, pp),
                        skip_group_check=True,
                    )
                st[Q] = {"psH": psH}

            def s1(Q):
                psH = st[Q].pop("psH")
                Ht = hp.tile([128, 2 * CH], BF16, name="H", tag="H")
                h_evac(Ht, psH)
                st[Q]["H"] = Ht

            def s2(Q):
                Ht = st[Q].pop("H")
                psZ = pszp.tile([128, 2 * CH], F32, name="psZ", tag="psZ")
                nc.tensor.matmul(
                    psZ[0:64, 0:CH], w2t[0:64], Ht[0:64, 0:CH],
                    start=True, stop=True, tile_position=(0, 0),
                    skip_group_check=True,
                )
                nc.tensor.matmul(
                    psZ[64:128, 0:CH], w2t[64:128], Ht[64:128, 0:CH],
                    start=True, stop=True, tile_position=(64, 64),
                    skip_group_check=True,
                )
                nc.tensor.matmul(
                    psZ[64:128, CH : 2 * CH], w2t[0:64], Ht[0:64, CH : 2 * CH],
                    start=True, stop=True, tile_position=(0, 64),
                    skip_group_check=True,
                )
                nc.tensor.matmul(
                    psZ[0:64, CH : 2 * CH], w2t[64:128], Ht[64:128, CH : 2 * CH],
                    start=True, stop=True, tile_position=(64, 0),
                    skip_group_check=True,
                )
                At = ap_.tile([128, 2 * CH], BF16, name="A", tag="A")
                nc.scalar.activation(At, psZ, AF.Tanh, bias=b2t)
                st[Q]["A"] = At

            chunk_acc = []

            def s3(Q):
                ck, q = Q // NQUAD, Q % NQUAD
                At = st.pop(Q)["A"]
                chunk_acc.append(At)
                if q == NQUAD - 1:
                    l3_pending.append([ck, list(chunk_acc), None, 0])
                    chunk_acc.clear()

            def drain_l3(budget):
                if not l3_pending:
                    return
                ent = l3_pending[0]
                ck, tiles, psY, emitted = ent
                if psY is None:
                    psY = pshp.tile([128, CH], F32, name="psY", tag="psH")
                    ent[2] = psY
                n = 0
                while n < budget and ent[3] < 2 * NQUAD:
                    p = ent[3]
                    q, half = p // 2, p % 2
                    c, j = p % 4, p // 4
                    nc.tensor.matmul(
                        psY[32 * c : 32 * c + 32, :],
                        w3t[j],
                        tiles[q][:, CH * half : CH * half + CH],
                        start=(p < 4),
                        stop=(p == 2 * NQUAD - 1),
                        tile_position=(0, 32 * c),
                        skip_group_check=True,
                    )
                    ent[3] += 1
                    n += 1
                if ent[3] == 2 * NQUAD:
                    ysb = youtp.tile([128, CH], F32, name="ysb", tag="ysb")
                    nc.vector.tensor_scalar(
                        ysb, psY, bot, None, mybir.AluOpType.add
                    )
                    nc.sync.dma_start(out=y[ck], in_=ysb)
                    l3_pending.pop(0)

            for t in range(NQ + 16):
                if t < NQ:
                    s0(t)
                if t == 2:
                    load_w3()
                if 0 <= t - 1 < NQ:
                    s1(t - 1)
                if 0 <= t - 2 < NQ:
                    s2(t - 2)
                drain_l3(L3_PER_ITER)
                if 0 <= t - 3 < NQ:
                    s3(t - 3)
            while l3_pending:
                drain_l3(L3_PER_ITER)
    _split_multiwaits(nc)
    return nc


def _to_bf16(a):
    """fp32 -> bf16 round-to-nearest-even, fast numpy bit trick."""
    u = np.ascontiguousarray(a, np.float32).view(np.uint32)
    r = ((u >> 16) & 1) + np.uint32(0x7FFF)
    return ((u + r) >> 16).astype(np.uint16).view(ml_dtypes.bfloat16)


def prep_weights(W_in, b_in, W_ih, b_ih, b_hh, W_out, b_out):
    W_in = np.asarray(W_in, np.float32)
    A = np.zeros((3, HID, 8), np.float32)
    A[0, :, 0:7] = W_in[:, 0:7] + W_in[:, 21:28]
    A[0, :, 7] = W_in[:, 28] + W_in[:, 31]
    A[1, :, 0:7] = W_in[:, 14:21]
    A[1, :, 7] = W_in[:, 30]
    A[2, :, 0:7] = W_in[:, 7:14]
    A[2, :, 7] = W_in[:, 29]

    w1b = np.zeros((32, HID), np.float32)
    for d in range(3):
        w1b[8 * d : 8 * d + 8] = A[d].T
    w1b[24] = np.asarray(b_in, np.float32)
    w1 = np.tile(w1b, (4, 1))  # [128, 64]

    w2 = np.tile(np.asarray(W_ih, np.float32).T, (2, 1))  # [128, 64]

    wo = np.asarray(W_out, np.float32)[0]  # [64]
    w3 = np.zeros((16, 128, 32), np.float32)
    for j in range(16):
        w3[j, 0:64, 2 * j] = wo
        w3[j, 64:128, 2 * j + 1] = wo

    bb = np.asarray(b_ih, np.float32) + np.asarray(b_hh, np.float32)
    b2 = np.concatenate([bb, bb]).reshape(128, 1).astype(np.float32)
    bo = np.full((128, 1), np.asarray(b_out, np.float32)[0], np.float32)
    return _to_bf16(w1), _to_bf16(w2), _to_bf16(w3), b2, bo


def prep_x_core(x, core):
    """x [4096,1024,8] f32 -> xq [32, 128, 4096] bf16 delay-stacked.

    Partition 32*b + row; rows 0-23: feature 8*(d-1)+c = x[t-d, g, c];
    row 24 = 1.0 (bias); rows 25-31 = 0.  g = 4*q + b.
    """
    xc = np.asarray(x[:, core * G_CORE : (core + 1) * G_CORE, :], np.float32)
    xg = np.ascontiguousarray(xc.transpose(1, 2, 0))  # [128, 8, T]
    out = np.zeros((NQUAD, 4, 32, T), np.float32)
    src = xg.reshape(NQUAD, 4, 8, T)
    for d in (1, 2, 3):
        out[:, :, 8 * (d - 1) : 8 * d, d:] = src[:, :, :, : T - d]
    out[:, :, 24, :] = 1.0
    return _to_bf16(out).reshape(NQUAD, 128, T)


def _basin_of_row():
    """Map psY row -> local basin index."""
    m = np.zeros(128, np.int64)
    for row in range(128):
        c, within = row // 32, row % 32
        j, e = within // 2, within % 2
        p = 4 * j + c
        q, half = p // 2, p % 2
        if half == 0:
            m[row] = 4 * q + e          # (A, B)
        else:
            m[row] = 4 * q + 3 - e      # (D, C)
    return m


_NC_CACHE = {}


def _get_nc():
    if "nc" not in _NC_CACHE:
        _NC_CACHE["nc"] = build_nc()
    return _NC_CACHE["nc"]


def kernel(x, W_in, b_in, W_ih, b_ih, W_hh, b_hh, W_out, b_out, _trace=False):
    from concourse.bass_utils import run_bass_kernel_spmd

    x = np.asarray(x, np.float32)
    w1, w2, w3, b2, bo = prep_weights(W_in, b_in, W_ih, b_ih, b_hh, W_out, b_out)
    in_maps = []
    for core in range(NCORES):
        in_maps.append(
            {
                "xq": prep_x_core(x, core),
                "w1": w1,
                "w2": w2,
                "w3": w3,
                "b2": b2,
                "bo": bo,
            }
        )
    nc = _get_nc()
    res = run_bass_kernel_spmd(nc, in_maps, list(range(NCORES)), trace=_trace)
    _NC_CACHE["last_result"] = res

    rowmap = _basin_of_row()
    out = np.empty((T, NG_ALL, 1), np.float32)
    out[:3, :, 0] = x[:3, :, 7]
    for core in range(NCORES):
        yc = res.results[core]["y"]  # [NCHUNK, 128, CH]
        yflat = yc.transpose(1, 0, 2).reshape(128, T)  # [row, t]
        g0 = core * G_CORE
        out[3:, g0 + rowmap, 0] = yflat[:, 3:].T
    return out


# revision 28
# speedup vs baseline: 1.2537x; 1.0154x over previous
"""Trainium2 Bass kernel for nn_NestedNarx: batched NARX MLP over basins.

Math (from the reference), for t >= 3:
  h = relu(W_in xt + b_in)          xt = 24 distinct delayed features
  a = tanh(W_ih h + b_ih + b_hh)
  y = W_out a + b_out
t < 3: y = x[t, :, 7].

Design (v2, PE tile_position packing + bf16):
  - All matmul operands bf16; PSUM stays fp32 (TRN2).
  - L1: per 4-basin "quad", four concurrent 32x64 tiled matmuls at
    positions (32r, 64*(r%2)); K rows = 24 features + ones row (bias
    folded into weights).  2 psH banks per quad, 2 basins each.
  - L2: 64x64 quadrant matmuls, 4 concurrent across 2 psZ halves of a
    [128,1024] two-bank tile (4 basins per tile).
  - L3: per basin-pair one 128x32 matmul (w_out placed at columns 2j,
    2j+1), 4 column-slots rotate -> one dense [128,512] psY bank per
    chunk holding y for all 128 basins x 512 t.
  - Evacuation (the bottleneck): DVE does relu for ~6/7 of psH tiles +
    psY bias; ACT does all tanh (FD=1024) + 1/7 of psH relu.

Sharding: pure data-parallel, 8 cores x 128 basins.
"""

import os
import sys

import numpy as np

for _p in ("/opt/trn_rl_repo",):
    if _p not in sys.path and os.path.isdir(_p):
        sys.path.insert(0, _p)

import ml_dtypes
import concourse.bass as bass
import concourse.mybir as mybir
from concourse.tile import TileContext

F32 = mybir.dt.float32
BF16 = mybir.dt.bfloat16
AF = mybir.ActivationFunctionType

T = 4096
NG_ALL = 1024
NCORES = 8
G_CORE = NG_ALL // NCORES  # 128 basins per core
NQUAD = 32                 # quads of 4 basins
CH = 512
NCHUNK = T // CH
HID = 64
ACT_H_EVERY = 19           # every 19th psH quad-tile evacuated on ScalarE


def _split_multiwaits(nc):
    """Single sem-wait per instruction (container walrus limitation)."""
    uid = [0]
    for fn in nc.m.functions:
        for bb in fn.blocks:
            new = []
            for inst in bb.instructions:
                si = inst.sync_info
                waits = list(si.on_wait) if si is not None and si.on_wait else []
                if len(waits) > 1:
                    for w in waits[:-1]:
                        uid[0] += 1
                        new.append(
                            mybir.InstNoOp(
                                name=f"{inst.name}-sw{uid[0]}",
                                engine=inst.engine,
                                bass_nofuse=True,
                                sync_info=mybir.SyncInfo(on_wait=[w], on_update=[]),
                            )
                        )
                    si.on_wait = waits[-1:]
                new.append(inst)
            bb.instructions = new


def build_nc():
    nc = bass.Bass()
    xq = nc.declare_dram_parameter("xq", [NQUAD, 128, T], BF16, isOutput=False)
    w1 = nc.declare_dram_parameter("w1", [128, HID], BF16, isOutput=False)
    w2 = nc.declare_dram_parameter("w2", [128, HID], BF16, isOutput=False)
    w3 = nc.declare_dram_parameter("w3", [16, 128, 32], BF16, isOutput=False)
    b2 = nc.declare_dram_parameter("b2", [128, 1], F32, isOutput=False)
    bo = nc.declare_dram_parameter("bo", [128, 1], F32, isOutput=False)
    y = nc.declare_dram_parameter("y", [NCHUNK, 128, CH], F32, isOutput=True)

    with TileContext(nc) as tc:
        with (
            tc.tile_pool(name="const", bufs=1) as constp,
            tc.tile_pool(name="xs", bufs=6) as xsp,
            tc.tile_pool(name="hh", bufs=8) as hp,
            tc.tile_pool(name="aa", bufs=44) as ap_,
            tc.tile_pool(name="yout", bufs=2) as youtp,
            tc.tile_pool(name="psh", bufs=2, space=bass.MemorySpace.PSUM) as pshp,
            tc.tile_pool(name="psz", bufs=2, space=bass.MemorySpace.PSUM) as pszp,
        ):
            # critical-path weights first so the first quads start ASAP;
            # w3 (L3-only, needed 3+ stages later) trails behind.
            w1t = constp.tile([128, HID], BF16, name="w1t")
            nc.sync.dma_start(out=w1t, in_=w1[:])
            w2t = constp.tile([128, HID], BF16, name="w2t")
            nc.sync.dma_start(out=w2t, in_=w2[:])
            b2t = constp.tile([128, 1], F32, name="b2t")
            nc.sync.dma_start(out=b2t, in_=b2[:])
            bot = constp.tile([128, 1], F32, name="bot")
            nc.sync.dma_start(out=bot, in_=bo[:])
            w3t = {}

            def load_w3(j0, j1):
                for j in range(j0, min(j1, 16)):
                    tl = constp.tile([128, 32], BF16, name=f"w3_{j}")
                    nc.sync.dma_start(out=tl, in_=w3[j])
                    w3t[j] = tl

            hctr = [0]

            def h_evac(Ht, psH):
                if hctr[0] % ACT_H_EVERY == ACT_H_EVERY - 1:
                    nc.scalar.activation(Ht, psH, AF.Relu)
                else:
                    nc.vector.tensor_scalar(
                        Ht, psH, 0.0, None, mybir.AluOpType.max
                    )
                hctr[0] += 1

            # Software-pipelined stages over global quad index Q:
            #   S0(Q): DMA xs + L1 -> psH
            #   S1(Q): relu evac psH -> H (DVE / every-7th ACT)
            #   S2(Q): L2 quadrants -> psZ, tanh -> A (ACT)
            #   S3(Q): collect A tiles; chunk's L3 runs as a deferred burst
            #          during the next chunk, psY borrowed from the psh pool
            NQ = NCHUNK * NQUAD
            st = {}
            l3_pending = []   # [ck, items, psY, emitted, done_at]
            l3_final = []     # [ck, psY, due_iter]
            L3_PER_ITER = 22

            # L1 slot layout: 4 basins in one [128,1024] 2-bank psH tile
            #   r=0 (0,0)   -> parts 0:64,   cols 0:CH
            #   r=1 (32,64) -> parts 64:128, cols 0:CH
            #   r=2 (64,0)  -> parts 0:64,   cols CH:2CH
            #   r=3 (96,64) -> parts 64:128, cols CH:2CH
            def s0(Q):
                ck, q = Q // NQUAD, Q % NQUAD
                t0 = ck * CH
                xs = xsp.tile([128, CH], BF16, name="xs", tag="xs")
                nc.sync.dma_start(out=xs, in_=xq[q][:, t0 : t0 + CH])
                psH = pshp.tile([128, 2 * CH], F32, name="psH", tag="psH")
                for r in range(4):
                    pp = 64 * (r % 2)
                    cc = CH * (r // 2)
                    nc.tensor.matmul(
                        psH[pp : pp + 64, cc : cc + CH],
                        w1t[32 * r : 32 * r + 32, :],
                        xs[32 * r : 32 * r + 32, :],
                        start=True,
                        stop=True,
                        tile_position=(32 * r, pp),
                        skip_group_check=True,
                    )
                st[Q] = {"psH": psH}

            def s1(Q):
                psH = st[Q].pop("psH")
                Ht = hp.tile([128, 2 * CH], BF16, name="H", tag="H")
                h_evac(Ht, psH)
                st[Q]["H"] = Ht

            def s2(Q):
                Ht = st[Q].pop("H")
                psZ = pszp.tile([128, 2 * CH], F32, name="psZ", tag="psZ")
                nc.tensor.matmul(
                    psZ[0:64, 0:CH], w2t[0:64], Ht[0:64, 0:CH],
                    start=True, stop=True, tile_position=(0, 0),
                    skip_group_check=True,
                )
                nc.tensor.matmul(
                    psZ[64:128, 0:CH], w2t[64:128], Ht[64:128, 0:CH],
                    start=True, stop=True, tile_position=(64, 64),
                    skip_group_check=True,
                )
                nc.tensor.matmul(
                    psZ[64:128, CH : 2 * CH], w2t[0:64], Ht[0:64, CH : 2 * CH],
                    start=True, stop=True, tile_position=(0, 64),
                    skip_group_check=True,
                )
                nc.tensor.matmul(
                    psZ[0:64, CH : 2 * CH], w2t[64:128], Ht[64:128, CH : 2 * CH],
                    start=True, stop=True, tile_position=(64, 0),
                    skip_group_check=True,
                )
                At = ap_.tile([128, 2 * CH], BF16, name="A", tag="A")
                nc.scalar.activation(At, psZ, AF.Tanh, bias=b2t)
                st[Q]["A"] = At

            chunk_acc = []

            def s3(Q):
                ck, q = Q // NQUAD, Q % NQUAD
                At = st.pop(Q)["A"]
                chunk_acc.append(At)
                if q == NQUAD - 1:
                    l3_pending.append([ck, list(chunk_acc), None, 0, None])
                    chunk_acc.clear()

            def finalize_l3(t, force=False):
                while l3_final and (force or l3_final[0][2] <= t):
                    ck, psY, _ = l3_final.pop(0)
                    ysb = youtp.tile([128, CH], F32, name="ysb", tag="ysb")
                    nc.vector.tensor_scalar(
                        ysb, psY, bot, None, mybir.AluOpType.add
                    )
                    nc.sync.dma_start(out=y[ck], in_=ysb)

            def drain_l3(budget, t):
                if not l3_pending:
                    return
                ent = l3_pending[0]
                ck, tiles, psY, emitted, _ = ent
                if psY is None:
                    psY = pshp.tile([128, CH], F32, name="psY", tag="psH")
                    ent[2] = psY
                n = 0
                while n < budget and ent[3] < 2 * NQUAD:
                    p = ent[3]
                    q, half = p // 2, p % 2
                    c, j = p % 4, p // 4
                    nc.tensor.matmul(
                        psY[32 * c : 32 * c + 32, :],
                        w3t[j],
                        tiles[q][:, CH * half : CH * half + CH],
                        start=(p < 4),
                        stop=(p == 2 * NQUAD - 1),
                        tile_position=(0, 32 * c),
                        skip_group_check=True,
                    )
                    ent[3] += 1
                    n += 1
                if ent[3] == 2 * NQUAD:
                    l3_final.append([ck, psY, t + 2])
                    l3_pending.pop(0)

            for t in range(NQ + 16):
                finalize_l3(t)
                if t < NQ:
                    s0(t)
                if 2 <= t < 10:
                    load_w3(2 * (t - 2), 2 * (t - 2) + 2)
                if 0 <= t - 1 < NQ:
                    s1(t - 1)
                if 0 <= t - 2 < NQ:
                    s2(t - 2)
                drain_l3(L3_PER_ITER, t)
                if 0 <= t - 3 < NQ:
                    s3(t - 3)
            t = NQ + 16
            while l3_pending:
                drain_l3(L3_PER_ITER, t)
                t += 1
            finalize_l3(t, force=True)
    _split_multiwaits(nc)
    return nc


def _to_bf16(a):
    """fp32 -> bf16 round-to-nearest-even, fast numpy bit trick."""
    u = np.ascontiguousarray(a, np.float32).view(np.uint32)
    r = ((u >> 16) & 1) + np.uint32(0x7FFF)
    return ((u + r) >> 16).astype(np.uint16).view(ml_dtypes.bfloat16)


def prep_weights(W_in, b_in, W_ih, b_ih, b_hh, W_out, b_out):
    W_in = np.asarray(W_in, np.float32)
    A = np.zeros((3, HID, 8), np.float32)
    A[0, :, 0:7] = W_in[:, 0:7] + W_in[:, 21:28]
    A[0, :, 7] = W_in[:, 28] + W_in[:, 31]
    A[1, :, 0:7] = W_in[:, 14:21]
    A[1, :, 7] = W_in[:, 30]
    A[2, :, 0:7] = W_in[:, 7:14]
    A[2, :, 7] = W_in[:, 29]

    w1b = np.zeros((32, HID), np.float32)
    for d in range(3):
        w1b[8 * d : 8 * d + 8] = A[d].T
    w1b[24] = np.asarray(b_in, np.float32)
    w1 = np.tile(w1b, (4, 1))  # [128, 64]

    w2 = np.tile(np.asarray(W_ih, np.float32).T, (2, 1))  # [128, 64]

    wo = np.asarray(W_out, np.float32)[0]  # [64]
    w3 = np.zeros((16, 128, 32), np.float32)
    for j in range(16):
        w3[j, 0:64, 2 * j] = wo
        w3[j, 64:128, 2 * j + 1] = wo

    bb = np.asarray(b_ih, np.float32) + np.asarray(b_hh, np.float32)
    b2 = np.concatenate([bb, bb]).reshape(128, 1).astype(np.float32)
    bo = np.full((128, 1), np.asarray(b_out, np.float32)[0], np.float32)
    return _to_bf16(w1), _to_bf16(w2), _to_bf16(w3), b2, bo


def prep_x_core(x, core):
    """x [4096,1024,8] f32 -> xq [32, 128, 4096] bf16 delay-stacked.

    Partition 32*b + row; rows 0-23: feature 8*(d-1)+c = x[t-d, g, c];
    row 24 = 1.0 (bias); rows 25-31 = 0.  g = 4*q + b.
    """
    xc = np.asarray(x[:, core * G_CORE : (core + 1) * G_CORE, :], np.float32)
    xg = np.ascontiguousarray(xc.transpose(1, 2, 0))  # [128, 8, T]
    out = np.zeros((NQUAD, 4, 32, T), np.float32)
    src = xg.reshape(NQUAD, 4, 8, T)
    for d in (1, 2, 3):
        out[:, :, 8 * (d - 1) : 8 * d, d:] = src[:, :, :, : T - d]
    out[:, :, 24, :] = 1.0
    return _to_bf16(out).reshape(NQUAD, 128, T)


def _basin_of_row():
    """Map psY row -> local basin index."""
    m = np.zeros(128, np.int64)
    for row in range(128):
        c, within = row // 32, row % 32
        j, e = within // 2, within % 2
        p = 4 * j + c
        q, half = p // 2, p % 2
        if half == 0:
            m[row] = 4 * q + e          # (A, B)
        else:
            m[row] = 4 * q + 3 - e      # (D, C)
    return m


_NC_CACHE = {}


def _get_nc():
    if "nc" not in _NC_CACHE:
        _NC_CACHE["nc"] = build_nc()
    return _NC_CACHE["nc"]


def kernel(x, W_in, b_in, W_ih, b_ih, W_hh, b_hh, W_out, b_out, _trace=False):
    from concourse.bass_utils import run_bass_kernel_spmd

    x = np.asarray(x, np.float32)
    w1, w2, w3, b2, bo = prep_weights(W_in, b_in, W_ih, b_ih, b_hh, W_out, b_out)
    in_maps = []
    for core in range(NCORES):
        in_maps.append(
            {
                "xq": prep_x_core(x, core),
                "w1": w1,
                "w2": w2,
                "w3": w3,
                "b2": b2,
                "bo": bo,
            }
        )
    nc = _get_nc()
    res = run_bass_kernel_spmd(nc, in_maps, list(range(NCORES)), trace=_trace)
    _NC_CACHE["last_result"] = res

    rowmap = _basin_of_row()
    out = np.empty((T, NG_ALL, 1), np.float32)
    out[:3, :, 0] = x[:3, :, 7]
    for core in range(NCORES):
        yc = res.results[core]["y"]  # [NCHUNK, 128, CH]
        yflat = yc.transpose(1, 0, 2).reshape(128, T)  # [row, t]
        g0 = core * G_CORE
        out[3:, g0 + rowmap, 0] = yflat[:, 3:].T
    return out
